# revision 1
# baseline (speedup 1.0000x reference)
"""Trainium2 Bass kernel for nn_Attention_37855841747487.

Dense transformer attention block: QKV projection, per-head L2-norm with
gamma * sqrt(d), xPos rotary embedding, GQA softmax attention (16 q heads,
4 kv heads), output projection with residual + bias.

Sharding: 8 cores = 2 batches x 4 query-row slices of 512. Each core
computes K/V for its full batch (duplicated across the 4 cores of that
batch) and attention + output projection for its 512 query rows. No
collectives.

On-core layout: projections contract over dim=1024 (x^T built via
SBUF->SBUF bf16 DMA-transpose), norm+rope run in natural [token, dim]
layout on DVE/ACT (norm commutes with rope, shortening the chain),
attention uses transposed scores S^T[keys, q] so softmax needs no
partition reductions: logits are bounded (l2-normalized q/k) so no max
pass is needed; the denominator comes from a ones-column appended to V.
Softmax exp is split between ScalarE (exact) and VectorE (Schraudolph
fast-exp: one fused multiply-add whose int16 result bit-pattern IS the
bf16 exp; softmax cancels the correlated approximation error). All
matmuls bf16 with fp32 PSUM accumulation. Measured: rel l2 error 2.3e-4
vs fp64 reference, ~235-280us/core on HW (differential in-NEFF repeat
timing; cost model predicts 280us).
"""

import sys

sys.path.insert(0, "/opt/trn_rl_repo")

import math

import numpy as np

B, N, DIM = 2, 2048, 1024
H, KVH, D = 16, 4, 64
XPOS_SB = 4096
QS = N // 4  # query rows per core
NCORES = 8

_CACHE = {}


# ---------------------------------------------------------------- host tables
def _make_tables(positions, scale_pow, gamma):
    """xPos rotary tables with rotate-half sign, gamma and rms folded in.

    Returns cosT, sinT of shape [n, Hg, 64]:
      roped(x) = l2norm(x) * cosT + swap_halves(l2norm(x)) * sinT
    where swap_halves swaps d<32 and d>=32.
    """
    d = D
    half = np.arange(0, d, 2, dtype=np.float64)
    inv_freq = 1.0 / (10000.0 ** (half / d))
    t = positions.astype(np.float64)
    freqs = t[:, None] * inv_freq[None, :]
    freqs = np.concatenate([freqs, freqs], axis=-1)
    base_scale = (half + 0.4 * d) / (1.4 * d)
    power = (t - N // 2) / XPOS_SB
    scale = base_scale[None, :] ** power[:, None]
    scale = np.concatenate([scale, scale], axis=-1)
    scale = scale**scale_pow
    cos = np.cos(freqs) * scale
    sin = np.sin(freqs) * scale
    sinA = np.concatenate([-sin[:, :32], sin[:, 32:]], axis=-1)
    rms = np.sqrt(np.float64(D))
    gswap = np.concatenate([gamma[:, 32:], gamma[:, :32]], axis=-1)
    cosT = cos[:, None, :] * (gamma[None, :, :] * rms)
    sinT = sinA[:, None, :] * (gswap[None, :, :] * rms)
    return cosT.astype(np.float32), sinT.astype(np.float32)


# ---------------------------------------------------------------- bass kernel
def _build_nc(ht, htk, repeat=1):
    """Trace + compile the per-core program. ht/htk: table head dims (1 when
    gamma is all-ones and the head axis broadcasts, else H / KVH)."""
    import concourse.bacc as bacc
    import concourse.bass as bass
    import concourse.mybir as mybir
    import concourse.tile as tile
    from concourse.masks import make_identity

    f32 = mybir.dt.float32
    bf16 = mybir.dt.bfloat16
    AF = mybir.ActivationFunctionType
    AX = mybir.AxisListType
    OP = mybir.AluOpType

    nc = bacc.Bacc("TRN2", target_bir_lowering=False, debug=False,
                   num_devices=NCORES, num_swdge_queues=4)

    xb_d = nc.dram_tensor("xb", [N, DIM], bf16, kind="ExternalInput")
    qxb_d = nc.dram_tensor("qxb", [QS, DIM], bf16, kind="ExternalInput")
    qx_d = nc.dram_tensor("qx", [QS, DIM], f32, kind="ExternalInput")
    wq_d = nc.dram_tensor("wq", [DIM, H * D], bf16, kind="ExternalInput")
    wkv_d = nc.dram_tensor("wkv", [DIM, 2 * KVH * D], bf16, kind="ExternalInput")
    wo_d = nc.dram_tensor("wo", [H * D, DIM], bf16, kind="ExternalInput")
    bo_d = nc.dram_tensor("bo", [DIM], f32, kind="ExternalInput")
    tq_dt = bf16
    tk_dt = bf16
    tqc_d = nc.dram_tensor("tqc", [QS, ht, D], tq_dt, kind="ExternalInput")
    tqs_d = nc.dram_tensor("tqs", [QS, ht, D], tq_dt, kind="ExternalInput")
    tkc_d = nc.dram_tensor("tkc", [N, htk, D], tk_dt, kind="ExternalInput")
    tks_d = nc.dram_tensor("tks", [N, htk, D], tk_dt, kind="ExternalInput")
    y_d = nc.dram_tensor("y", [QS, DIM], f32, kind="ExternalOutput")

    from contextlib import ExitStack

    with tile.TileContext(nc) as tc, ExitStack() as ctx:
        persist = ctx.enter_context(tc.tile_pool(name="persist", bufs=1))
        dram = ctx.enter_context(tc.tile_pool(name="dram", bufs=1, space="DRAM"))
        stage = ctx.enter_context(tc.tile_pool(name="stage", bufs=4))

        # ---- persistent SBUF tensors
        wq_sb = persist.tile([128, 8, H * D], bf16)
        wkv_sb = persist.tile([128, 8, 2 * KVH * D], bf16)
        qxT_sb = persist.tile([128, 8, QS], bf16)     # qx^T
        qT_sb = persist.tile([128, 8, QS], bf16)      # roped q^T
        kT_sb = persist.tile([128, 2, N], bf16)       # roped k^T
        v_sb = persist.tile([128, 16, KVH * (D + 1)], bf16)  # v natural + ones
        aoT_sb = persist.tile([128, 8, QS], bf16)     # attention out^T
        tqc_sb = persist.tile([128, 4, ht, D], tq_dt)
        tqs_sb = persist.tile([128, 4, ht, D], tq_dt)
        tkc_sb = persist.tile([128, 16, htk, D], tk_dt)
        tks_sb = persist.tile([128, 16, htk, D], tk_dt)
        bo_sb = persist.tile([128, 8], f32)
        ident = persist.tile([128, 128], f32)
        ident_bf = persist.tile([128, 128], bf16)
        ones1 = persist.tile([1, D], f32)
        make_identity(nc, ident)
        make_identity(nc, ident_bf)
        nc.vector.memset(ones1, 1.0)
        v4 = v_sb.rearrange("p a (kv e) -> p a kv e", e=D + 1)
        nc.vector.memset(v4[:, :, :, D : D + 1], 1.0)
        # commit the persistent pool's layout before any scoped pool opens
        persist.seal()

        for _rep in range(repeat):
            # ---- A0: natural fp32 loads -> DVE bf16 cast -> SBUF->SBUF
            # DMA-transpose per 128-token tile; weights stream via SWDGE-cast
            # DMAs in parallel; wo deferred to phase B/C.
            qxf_ctx = tc.tile_pool(name="qxf_pool", bufs=1)
            qxf_pool = qxf_ctx.__enter__()
            qxf_sb = qxf_pool.tile([128, 4, DIM], f32)  # qx natural (residual)
            qxf_pool.seal()
            xT_ctx = tc.tile_pool(name="xT_pool", bufs=1)
            xT_pool = xT_ctx.__enter__()
            xT_sb = xT_pool.tile([128, 8, N], bf16)       # x_b^T
            xT_pool.seal()
            nc.sync.dma_start(out=tkc_sb, in_=tkc_d.rearrange("(a p) h d -> p a h d", p=128))
            nc.sync.dma_start(out=tks_sb, in_=tks_d.rearrange("(a p) h d -> p a h d", p=128))
            nc.sync.dma_start(out=tqc_sb, in_=tqc_d.rearrange("(a p) h d -> p a h d", p=128))
            nc.sync.dma_start(out=tqs_sb, in_=tqs_d.rearrange("(a p) h d -> p a h d", p=128))
            nc.sync.dma_start(out=bo_sb, in_=bo_d.rearrange("(m p) -> p m", p=128))
            for kt in range(8):
                nc.sync.dma_start(out=wkv_sb[:, kt, :],
                                  in_=wkv_d[kt * 128 : (kt + 1) * 128, :])
            # x^T / qx^T: the host ships x already in bf16, so the xbar
            # DMA-transpose reads the DRAM input directly -- no staging, no
            # PE transposes. 512-row chunks so the kv projection pipeline
            # starts as soon as chunk 0 lands.
            for c in range(4):
                nc.sync.dma_start_transpose(
                    out=xT_sb[:, :, c * 512 : (c + 1) * 512],
                    in_=xb_d[c * 512 : (c + 1) * 512, :])
            nc.sync.dma_start_transpose(out=qxT_sb, in_=qxb_d[:, :])
            if True:
                for kt in range(8):
                    nc.sync.dma_start(out=wq_sb[:, kt, :],
                                      in_=wq_d[kt * 128 : (kt + 1) * 128, :])

            def norm_rope(pin, cos_t, sin_t, hout, A, Hh):
                """pin: PSUM fp32 [128, A, Hh, 64] projected tile (natural
                layout). cos_t/sin_t: bf16 [128, A, Hh, 64] step-1 table APs.
                hout: SBUF bf16 [128, A, Hh, 64] roped, normalized output.
                rope(l2norm(x)) == rope(x)/||x||, so the sum-of-squares chain
                (fp32, from PSUM) runs in parallel with the rope multiplies
                (bf16 at DVE 2x, from a cast copy) and joins at the end."""
                sq = stage.tile([128, A, Hh, D], f32, tag="sq")
                nc.scalar.activation(sq, pin, AF.Square)
                ss = stage.tile([128, A, Hh], f32, tag="ss")
                nc.vector.tensor_reduce(ss, sq, axis=AX.X, op=OP.add)
                nrm = stage.tile([128, A, Hh], f32, tag="nrm")
                nc.scalar.activation(nrm, ss, AF.Sqrt)
                rs = stage.tile([128, A, Hh], f32, tag="rs")
                nc.vector.reciprocal(rs, nrm)
                rsb = rs.unsqueeze(3).broadcast_to([128, A, Hh, D])
                pb = stage.tile([128, A, Hh, D], bf16, tag="pb")
                nc.scalar.copy(out=pb, in_=pin)
                r1 = stage.tile([128, A, Hh, D], bf16, tag="t1")
                nc.vector.tensor_tensor(out=r1, in0=pb, in1=cos_t, op=OP.mult)
                nc.vector.tensor_tensor(out=hout[:, :, :, 0:32],
                                        in0=pb[:, :, :, 32:64],
                                        in1=sin_t[:, :, :, 0:32], op=OP.mult)
                nc.vector.tensor_tensor(out=hout[:, :, :, 32:64],
                                        in0=pb[:, :, :, 0:32],
                                        in1=sin_t[:, :, :, 32:64], op=OP.mult)
                nc.vector.tensor_tensor(out=hout, in0=hout, in1=r1, op=OP.add)
                nc.vector.tensor_tensor(out=hout, in0=hout, in1=rsb, op=OP.mult)

            # ---- A1: merged k|v projection + q projection in ONE psum scope
            # (kv 4 banks + q 2 banks + shared transpose pool 2 banks = 8) so
            # the q pipeline overlaps the k norm/rope chain tail.
            with tc.tile_pool(name="kv_ps", bufs=2, space="PSUM") as kv_ps, \
                 tc.tile_pool(name="q_ps", bufs=2, space="PSUM") as q_ps, \
                 tc.tile_pool(name="ktp", bufs=2, space="PSUM") as ktp_ps:
                qtp_ps = ktp_ps
                for g in range(8):
                    kvp = kv_ps.tile([128, 2, 2 * KVH * D], f32)
                    for i in range(2):
                        mt = g * 2 + i
                        for kt in range(8):
                            nc.tensor.matmul(
                                kvp[:, i, :],
                                lhsT=xT_sb[:, kt, mt * 128 : (mt + 1) * 128],
                                rhs=wkv_sb[:, kt, :],
                                start=(kt == 0), stop=(kt == 7))
                    kv8 = kvp.rearrange("p a (g2 d) -> p a g2 d", d=D)
                    # v evacuation into 65-column blocks (ones col pre-set)
                    nc.scalar.copy(
                        out=v4[:, g * 2 : (g + 1) * 2, :, 0:D],
                        in_=kv8[:, :, KVH : 2 * KVH, :])
                    # k: norm + rope over the 2 m-tiles at once
                    khat = stage.tile([128, 2, KVH, D], bf16, tag="hat")
                    norm_rope(kv8[:, :, 0:KVH, :],
                              tkc_sb[:, g * 2 : (g + 1) * 2],
                              tks_sb[:, g * 2 : (g + 1) * 2], khat, 2, KVH)
                    kflat = khat.rearrange("p a h d -> p a (h d)")
                    for i in range(2):
                        mt = g * 2 + i
                        tp = ktp_ps.tile([128, 4, 128], bf16, tag="tp")
                        for c in range(2):
                            nc.tensor.transpose(tp[:, c, :],
                                                kflat[:, i, c * 128 : (c + 1) * 128],
                                                ident_bf)
                        nc.scalar.copy(out=kT_sb[:, :, mt * 128 : (mt + 1) * 128],
                                       in_=tp[:, 0:2, :])

                # ---- A1-Q: q projection in half-tiles (8 heads each -> one
                # PSUM bank) + norm + rope + transpose to qT_sb
                for nn in range(2):
                    for m in range(4):
                        qp = q_ps.tile([128, 512], f32)
                        for kt in range(8):
                            nc.tensor.matmul(
                                qp,
                                lhsT=qxT_sb[:, kt, m * 128 : (m + 1) * 128],
                                rhs=wq_sb[:, kt, nn * 512 : (nn + 1) * 512],
                                start=(kt == 0), stop=(kt == 7))
                        qhat = stage.tile([128, 1, H // 2, D], bf16, tag="hat")
                        qin = qp.rearrange("p (o h d) -> p o h d", o=1, d=D)
                        norm_rope(qin,
                                  tqc_sb[:, m, nn * 8 : (nn + 1) * 8].unsqueeze(1),
                                  tqs_sb[:, m, nn * 8 : (nn + 1) * 8].unsqueeze(1),
                                  qhat, 1, H // 2)
                        qflat = qhat.rearrange("p o h d -> p (o h d)")
                        tp = qtp_ps.tile([128, 4, 128], bf16, tag="tp")
                        for j4 in range(4):
                            nc.tensor.transpose(tp[:, j4, :],
                                                qflat[:, j4 * 128 : (j4 + 1) * 128],
                                                ident_bf)
                        nc.scalar.copy(
                            out=qT_sb[:, nn * 4 : (nn + 1) * 4, m * 128 : (m + 1) * 128],
                            in_=tp)
            xT_ctx.__exit__(None, None, None)

            # ---- B: attention per head (wo weights stream in concurrently).
            # Softmax exp is split between ScalarE (exact spline exp) and
            # VectorE (Schraudolph fast-exp: one fused multiply-add whose
            # int16-converted result IS the bf16 bit pattern of exp(x/8);
            # softmax's shared denominator cancels most of the correlated
            # approximation error -- validated end-to-end at ~2e-4 rel).
            FE_A = 16.0 / math.log(2.0)
            FE_B = 127.0 * 128.0 - 366000.0 / 65536.0
            wo_ctx = tc.tile_pool(name="wo_pool", bufs=1)
            wo_pool = wo_ctx.__enter__()
            wo_sb = wo_pool.tile([128, 8, DIM], bf16)
            wo_pool.seal()
            for kt in range(8):
                nc.sync.dma_start(out=wo_sb[:, kt, :],
                                  in_=wo_d[kt * 128 : (kt + 1) * 128, :])
            nc.sync.dma_start(out=qxf_sb,
                              in_=qx_d.rearrange("(a p) d -> p a d", p=128))
            groups = [(i, i + 1) for i in range(16)]
            with tc.tile_pool(name="sT_ps", bufs=6, space="PSUM") as sT_ps, \
                 tc.tile_pool(name="oT_ps", bufs=2, space="PSUM") as oT_ps, \
                 tc.tile_pool(name="pT_pool", bufs=6) as pT_pool, \
                 tc.tile_pool(name="small", bufs=3) as small:
                for h in range(H):
                    kvh = h % KVH
                    jq, qp_off = h // 2, 64 * (h % 2)
                    ktile, kp_off = kvh // 2, 64 * (kvh % 2)
                    oT = oT_ps.tile([D + 1, 512], f32)
                    pending = None
                    def do_av(pT, a, b):
                        for i, kt in enumerate(range(a, b)):
                            nc.tensor.matmul(
                                oT,
                                lhsT=v_sb[:, kt, kvh * (D + 1) : (kvh + 1) * (D + 1)],
                                rhs=pT[:, i, :],
                                start=(kt == 0), stop=(kt == 15))
                    for gi, (a, b) in enumerate(groups):
                        ng = b - a
                        sT = sT_ps.tile([128, 1, 512], f32)
                        for i, kt in enumerate(range(a, b)):
                            nc.tensor.matmul(
                                sT[:, i, :],
                                lhsT=kT_sb[kp_off : kp_off + 64, ktile,
                                           kt * 128 : (kt + 1) * 128],
                                rhs=qT_sb[qp_off : qp_off + 64, jq, :],
                                start=True, stop=True)
                        pT = pT_pool.tile([128, 1, 512], bf16)
                        on_dve = gi % 5 in (1, 3)
                        if on_dve:
                            nc.vector.tensor_scalar(
                                out=pT[:, 0:ng, :].bitcast(mybir.dt.int16),
                                in0=sT[:, 0:ng, :],
                                scalar1=FE_A, scalar2=FE_B,
                                op0=OP.mult, op1=OP.add)
                        else:
                            nc.scalar.activation(pT[:, 0:ng, :], sT[:, 0:ng, :],
                                                 AF.Exp, scale=0.125)
                        if pending is not None:
                            do_av(*pending)
                        pending = (pT, a, b)
                    do_av(*pending)
                    recip = small.tile([1, 512], f32, tag="recip")
                    nc.vector.reciprocal(recip, oT[D : D + 1, :])
                    rb = small.tile([D, 512], f32, tag="rb")
                    nc.gpsimd.partition_broadcast(rb, recip)
                    nc.vector.tensor_tensor(
                        out=aoT_sb[qp_off : qp_off + 64, jq, :],
                        in0=oT[0:D, :], in1=rb, op=OP.mult)

            # ---- C: output projection + bias + transpose + residual + store
            with tc.tile_pool(name="y_ps", bufs=2, space="PSUM") as y_ps, \
                 tc.tile_pool(name="otp", bufs=2, space="PSUM") as otp_ps, \
                 tc.tile_pool(name="cstage", bufs=1) as cstage, \
                 tc.tile_pool(name="ystage", bufs=2) as ystage:
                y1_sb = cstage.tile([128, 8, QS], f32)     # y^T before final transpose
                qxf_sb = cstage.tile([128, 4, DIM], f32)   # qx natural (residual)
                nc.sync.dma_start(out=qxf_sb, in_=qx_d.rearrange("(a p) d -> p a d", p=128))
                for m in range(8):
                    yp = y_ps.tile([128, 512], f32)
                    for kt in range(8):
                        nc.tensor.matmul(yp,
                                         lhsT=wo_sb[:, kt, m * 128 : (m + 1) * 128],
                                         rhs=aoT_sb[:, kt, :],
                                         start=(kt == 0), stop=(kt == 7))
                    nc.vector.tensor_scalar_add(y1_sb[:, m, :], in0=yp,
                                                scalar1=bo_sb[:, m : m + 1])
                for tq in range(4):
                    ot = otp_ps.tile([128, 8, 128], f32)
                    for m in range(8):
                        nc.tensor.transpose(ot[:, m, :],
                                            y1_sb[:, m, tq * 128 : (tq + 1) * 128],
                                            ident)
                    yn = ystage.tile([128, DIM], f32)
                    nc.vector.tensor_tensor(out=yn,
                                            in0=ot.rearrange("p a b -> p (a b)"),
                                            in1=qxf_sb[:, tq, :], op=OP.add)
                    nc.sync.dma_start(out=y_d[tq * 128 : (tq + 1) * 128, :], in_=yn)
            wo_ctx.__exit__(None, None, None)
            qxf_ctx.__exit__(None, None, None)

    nc.compile()
    return nc


def _get_nc(ht, htk, repeat=1):
    key = (ht, htk, repeat)
    if key not in _CACHE:
        _CACHE[key] = _build_nc(ht, htk, repeat)
    return _CACHE[key]


# ---------------------------------------------------------------- entry point
def make_in_maps(x, Wq, Wkv, q_gamma, k_gamma, Wo, bo):
    import ml_dtypes
    bf = ml_dtypes.bfloat16
    x = np.ascontiguousarray(np.asarray(x, dtype=np.float32))
    x16 = x.astype(bf)
    Wq = np.ascontiguousarray(np.asarray(Wq, dtype=np.float32).astype(bf))
    Wkv = np.ascontiguousarray(np.asarray(Wkv, dtype=np.float32).astype(bf))
    Wo = np.ascontiguousarray(np.asarray(Wo, dtype=np.float32).astype(bf))
    bo = np.ascontiguousarray(np.asarray(bo, dtype=np.float32))
    qg = np.asarray(q_gamma, dtype=np.float64).reshape(H, D)
    kg = np.asarray(k_gamma, dtype=np.float64).reshape(KVH, D)

    ht, htk = H, KVH
    pos = np.arange(N)
    tkc, tks = _make_tables(pos, -1.0, kg)
    tkc, tks = tkc.astype(bf), tks.astype(bf)

    in_maps = []
    for c in range(NCORES):
        bi, qi = c // 4, c % 4
        qpos = pos[qi * QS : (qi + 1) * QS]
        tqc, tqs = _make_tables(qpos, +1.0, qg)
        tqc, tqs = tqc.astype(bf), tqs.astype(bf)
        in_maps.append({
            "xb": x16[bi],
            "qxb": np.ascontiguousarray(x16[bi, qi * QS : (qi + 1) * QS]),
            "qx": np.ascontiguousarray(x[bi, qi * QS : (qi + 1) * QS]),
            "wq": Wq, "wkv": Wkv, "wo": Wo, "bo": bo,
            "tqc": np.ascontiguousarray(tqc), "tqs": np.ascontiguousarray(tqs),
            "tkc": np.ascontiguousarray(tkc), "tks": np.ascontiguousarray(tks),
        })
    return in_maps, (ht, htk)


def kernel(x, Wq, Wkv, q_gamma, k_gamma, Wo, bo):
    from concourse import bass_utils

    in_maps, (ht, htk) = make_in_maps(x, Wq, Wkv, q_gamma, k_gamma, Wo, bo)
    nc = _get_nc(ht, htk)
    res = bass_utils.run_bass_kernel_spmd(nc, in_maps,
                                          core_ids=list(range(NCORES)))
    out = np.zeros((B, N, DIM), np.float32)
    for c in range(NCORES):
        bi, qi = c // 4, c % 4
        out[bi, qi * QS : (qi + 1) * QS] = res.results[c]["y"]
    return out



# revision 16
# speedup vs baseline: 2.3463x; 2.3463x over previous
"""Trainium2 Bass kernel for nn_Attention_37855841747487.

Dense transformer attention block: QKV projection, per-head L2-norm with
gamma * sqrt(d), xPos rotary embedding, GQA softmax attention (16 q heads,
4 kv heads), output projection with residual + bias.

Sharding: 8 cores = 2 batches x 4 query-row slices of 512. Each core
computes K/V for its full batch (duplicated across the 4 cores of that
batch) and attention + output projection for its 512 query rows. No
collectives.

v3 design (vs the bf16 v1 at ~345us):
- All projections (QKV, O) and attention@V run in fp8-e4m3 with DoubleRow
  perf mode (256-deep contraction per instruction: 2x bf16 PE array
  throughput AND half the instruction count -- the PE sequencer's ~174ns
  per-matmul decode is a first-order cost at this problem size). Weights
  are pre-scaled by 64 on the host so their 0.02-sigma values leave the
  e4m3 subnormal range; the scale cancels in l2norm for Q/K and is divided
  back out in the V-evacuation / Y-evacuation.
- Scores stay bf16 (contraction is d=64, so fp8 DoubleRow cannot shorten
  the instruction; the 16.8M-element score output bounds PE array time at
  one psum column per cycle regardless of dtype).
- Softmax exp is computed per 256-key block straight from PSUM into
  fp8-e5m2 probabilities in ONE op per block (constant scale/bias), split
  across three engines: ScalarE (exact Exp activation), VectorE and GpSimd
  (Schraudolph fast-exp: a fused multiply-add whose uint8 result
  bit-pattern IS the e5m2 exp; bits stay in [19, 101] for this data's
  score range so no saturation path is exercised, and softmax's shared
  denominator cancels the correlated piecewise-linear error).
- attention@V keeps the transposed S^T[keys, q] orientation (out [65, q]
  = [d | denominator-from-ones-column]): one DoubleRow instruction per
  (head, key-pair-block), and the result lands directly in the ao^T
  layout the output projection consumes -- no transposes in phase C.
- The output projection takes ao^T as the stationary operand and streams
  natural Wo, producing natural-layout Y directly: no fp32 transposes
  anywhere. bo is folded into the residual on the host.
Measured (numpy model of the full quantization pipeline): rel l2 err
~1.7e-3 vs fp64 reference.
"""

import sys

sys.path.insert(0, "/opt/trn_rl_repo")

import math
import os

import numpy as np

# phase-bisection knob for sim diagnostics only ('A', 'AB', or 'ABC')
PHASES = os.environ.get('K_PHASES', 'ABC')

B, N, DIM = 2, 2048, 1024
H, KVH, D = 16, 4, 64
XPOS_SB = 4096
QS = N // 4  # query rows per core
NCORES = 8

# Schraudolph fast-exp to fp8-e5m2 bit patterns: P = exp(S/8) -> bits =
# 4*(log2 P + 15) = S * 4/(8 ln2) + 60.
SCH_E5_SLOPE = 0.5 / math.log(2.0)
SCH_E5_BIAS = 60.0
ACT_EXP_SCALE = 0.125

# exp engine rotation per (head, kt-pair) slot: 6 DVE, 10 Act.
# (GpSimd cannot read PSUM on TRN2, so it only gets SBUF-side work: the
# division's partition_broadcast and the norm multiplies.)
ROT = ['A', 'D', 'A', 'A', 'D', 'A', 'A', 'D',
       'A', 'A', 'D', 'A', 'A', 'D', 'A', 'D']

_CACHE = {}


# ---------------------------------------------------------------- host tables
def _make_tables(positions, scale_pow, gamma):
    """xPos rotary tables with rotate-half sign, gamma and rms folded in.

    Returns cosT, sinT of shape [n, Hg, 64]:
      roped(x) = x * cosT + swap_halves(x) * sinT   (applied pre-norm; the
    1/||x|| multiply happens separately on chip).
    """
    d = D
    half = np.arange(0, d, 2, dtype=np.float64)
    inv_freq = 1.0 / (10000.0 ** (half / d))
    t = positions.astype(np.float64)
    freqs = t[:, None] * inv_freq[None, :]
    freqs = np.concatenate([freqs, freqs], axis=-1)
    base_scale = (half + 0.4 * d) / (1.4 * d)
    power = (t - N // 2) / XPOS_SB
    scale = base_scale[None, :] ** power[:, None]
    scale = np.concatenate([scale, scale], axis=-1)
    scale = scale**scale_pow
    rms = np.sqrt(np.float64(D))
    cos = np.cos(freqs) * scale * rms
    sin = np.sin(freqs) * scale * rms
    sinA = np.concatenate([-sin[:, :32], sin[:, 32:]], axis=-1)
    gswap = np.concatenate([gamma[:, 32:], gamma[:, :32]], axis=-1)
    cosT = cos[:, None, :] * gamma[None, :, :]
    sinT = sinA[:, None, :] * gswap[None, :, :]
    return cosT.astype(np.float32), sinT.astype(np.float32)


# ---------------------------------------------------------------- bass kernel
def _build_nc(ht, htk, repeat=1):
    """Trace + compile the per-core program. ht/htk: table head dims (1 when
    gamma is all-ones and the head axis broadcasts, else H / KVH)."""
    import concourse.bacc as bacc
    import concourse.bass as bass
    import concourse.mybir as mybir
    import concourse.tile as tile
    from concourse.masks import make_identity

    f32 = mybir.dt.float32
    bf16 = mybir.dt.bfloat16
    e4 = mybir.dt.float8e4
    e5 = mybir.dt.float8e5
    u8 = mybir.dt.uint8
    AF = mybir.ActivationFunctionType
    AX = mybir.AxisListType
    OP = mybir.AluOpType
    DR = mybir.MatmulPerfMode.DoubleRow

    nc = bacc.Bacc("TRN2", target_bir_lowering=False, debug=False,
                   num_devices=NCORES, num_swdge_queues=4)

    xT8_d = nc.dram_tensor("xT8", [128, 8, N], e4, kind="ExternalInput")
    qxT8_d = nc.dram_tensor("qxT8", [128, 8, QS], e4, kind="ExternalInput")
    wq8_d = nc.dram_tensor("wq8", [128, 8, H * D], e4, kind="ExternalInput")
    wkv8_d = nc.dram_tensor("wkv8", [128, 8, 2 * KVH * D], e4, kind="ExternalInput")
    wo8_d = nc.dram_tensor("wo8", [128, 8, DIM], e4, kind="ExternalInput")
    qxr_d = nc.dram_tensor("qxr", [128, 4, DIM], f32, kind="ExternalInput")
    tqc_d = nc.dram_tensor("tqc", [128, QS // 128, ht, D], bf16, kind="ExternalInput")
    tqs_d = nc.dram_tensor("tqs", [128, QS // 128, ht, D], bf16, kind="ExternalInput")
    tkc_d = nc.dram_tensor("tkc", [128, N // 128, htk, D], bf16, kind="ExternalInput")
    tks_d = nc.dram_tensor("tks", [128, N // 128, htk, D], bf16, kind="ExternalInput")
    y_d = nc.dram_tensor("y", [QS, DIM], f32, kind="ExternalOutput")

    from contextlib import ExitStack

    with tile.TileContext(nc) as tc, ExitStack() as ctx:
        persist = ctx.enter_context(tc.tile_pool(name="persist", bufs=1))
        stage = ctx.enter_context(tc.tile_pool(name="stage", bufs=4))

        # ---- persistent SBUF tensors
        wq_sb = persist.tile([128, 8, H * D], e4)
        wkv_sb = persist.tile([128, 8, 2 * KVH * D], e4)
        qT_sb = persist.tile([128, 8, QS], bf16)      # roped, normalized q^T
        kT_sb = persist.tile([128, 2, N], bf16)       # roped, normalized k^T
        v_sb = persist.tile([128, 16, KVH * 128], e4)  # v | ones | zero-pad
        aoT_sb = persist.tile([128, 8, QS], e4)       # attention out^T, fp8
        tqc_sb = persist.tile([128, QS // 128, ht, D], bf16)
        tqs_sb = persist.tile([128, QS // 128, ht, D], bf16)
        tkc_sb = persist.tile([128, N // 128, htk, D], bf16)
        tks_sb = persist.tile([128, N // 128, htk, D], bf16)
        ident_bf = persist.tile([128, 128], bf16)
        make_identity(nc, ident_bf)
        v4 = v_sb.rearrange("p a (kv e) -> p a kv e", e=128)
        nc.vector.memset(v4[:, :, :, D : D + 1], 1.0)
        nc.vector.memset(v4[:, :, :, D + 1 : 128], 0.0)
        persist.seal()

        for _rep in range(repeat):
            # ---- A0: stream weights + x^T (pre-transposed, pre-cast on host)
            xp_ctx = tc.tile_pool(name="x_pool", bufs=1)
            x_pool = xp_ctx.__enter__()
            xT_sb = x_pool.tile([128, 8, N], e4)
            qxT_sb = x_pool.tile([128, 8, QS], e4)
            x_pool.seal()
            nc.sync.dma_start(out=tkc_sb, in_=tkc_d[:, :, :, :])
            nc.sync.dma_start(out=tks_sb, in_=tks_d[:, :, :, :])
            nc.sync.dma_start(out=tqc_sb, in_=tqc_d[:, :, :, :])
            nc.sync.dma_start(out=tqs_sb, in_=tqs_d[:, :, :, :])
            nc.sync.dma_start(out=wkv_sb, in_=wkv8_d[:, :, :])
            for c in range(4):
                nc.sync.dma_start(out=xT_sb[:, :, c * 512 : (c + 1) * 512],
                                  in_=xT8_d[:, :, c * 512 : (c + 1) * 512])
            nc.sync.dma_start(out=qxT_sb, in_=qxT8_d[:, :, :])
            nc.sync.dma_start(out=wq_sb, in_=wq8_d[:, :, :])

            # ---- A1: projections, then BATCHED norm/rope epilogues.
            # Per-tile work is only the PSUM evacuation (Act bf16 copy + V
            # fp8 copy); the l2-norm and rope run as a handful of
            # whole-tensor ops (the per-call version cost ~200 small ops
            # and their cross-engine sync dominated the phase). k^T / q^T
            # are produced by xbar DMA-transposes instead of PE transposes.
            ap_ctx = tc.tile_pool(name="apool", bufs=1)
            apool = ap_ctx.__enter__()
            pbk = apool.tile([128, 16, KVH, D], bf16)
            khat = apool.tile([128, 16, KVH, D], bf16)
            r1k = apool.tile([128, 16, KVH, D], bf16)
            pbq = apool.tile([128, 2, 4, H // 2, D], bf16)
            qhat = apool.tile([128, 2, 4, H // 2, D], bf16)
            r1q = apool.tile([128, 2, 4, H // 2, D], bf16)
            ssk_sb = apool.tile([128, 16, KVH], f32)
            ssq_sb = apool.tile([128, 2, 4, H // 2], f32)
            apool.seal()
            with tc.tile_pool(name="kv_ps", bufs=2, space="PSUM") as kv_ps, \
                 tc.tile_pool(name="q_ps", bufs=2, space="PSUM") as q_ps:
                for g in range(8):
                    kvp = kv_ps.tile([128, 2, 2 * KVH * D], f32)
                    for i in range(2):
                        mt = g * 2 + i
                        for s in range(4):
                            nc.tensor.matmul(
                                kvp[:, i, :],
                                lhsT=xT_sb[:, 2 * s : 2 * s + 2,
                                           mt * 128 : (mt + 1) * 128],
                                rhs=wkv_sb[:, 2 * s : 2 * s + 2, :],
                                start=(s == 0), stop=(s == 3), perf_mode=DR)
                    kv8 = kvp.rearrange("p a (g2 d) -> p a g2 d", d=D)
                    nc.scalar.mul(v4[:, g * 2 : (g + 1) * 2, :, 0:D],
                                  kv8[:, :, KVH : 2 * KVH, :], 1.0 / 64.0)
                    nc.scalar.copy(out=pbk[:, g * 2 : (g + 1) * 2],
                                   in_=kv8[:, :, 0:KVH, :])
                    sqg = stage.tile([128, 2, KVH, D], f32, tag="sqg")
                    nc.scalar.activation(sqg, kv8[:, :, 0:KVH, :], AF.Square)
                    nc.vector.tensor_reduce(ssk_sb[:, g * 2 : (g + 1) * 2],
                                            sqg, axis=AX.X, op=OP.add)
                for nn in range(2):
                    for m in range(4):
                        qp = q_ps.tile([128, 512], f32)
                        for s in range(4):
                            nc.tensor.matmul(
                                qp,
                                lhsT=qxT_sb[:, 2 * s : 2 * s + 2,
                                            m * 128 : (m + 1) * 128],
                                rhs=wq_sb[:, 2 * s : 2 * s + 2,
                                          nn * 512 : (nn + 1) * 512],
                                start=(s == 0), stop=(s == 3), perf_mode=DR)
                        nc.scalar.copy(
                            out=pbq[:, nn, m],
                            in_=qp.rearrange("p (h d) -> p h d", d=D))
                        sqg = stage.tile([128, 1, H // 2, D], f32, tag="sqg")
                        qpv = qp.rearrange("p (o h d) -> p o h d", o=1, d=D)
                        nc.scalar.activation(sqg, qpv, AF.Square)
                        nc.vector.tensor_reduce(
                            ssq_sb[:, nn, m : m + 1], sqg,
                            axis=AX.X, op=OP.add)

                # ---- batched K epilogue: rope, 1/||k||, transpose
                nrmk = stage.tile([128, 16, KVH], f32, tag="nrmk")
                nc.scalar.activation(nrmk, ssk_sb, AF.Sqrt)
                rsk = stage.tile([128, 16, KVH], f32, tag="rsk")
                nc.vector.reciprocal(rsk, nrmk)
                if htk == 1:
                    ckN = tkc_sb.broadcast_to([128, 16, KVH, D])
                    skN = tks_sb.broadcast_to([128, 16, KVH, D])
                else:
                    ckN, skN = tkc_sb, tks_sb
                nc.vector.tensor_tensor(out=r1k, in0=pbk, in1=ckN, op=OP.mult)
                nc.vector.tensor_tensor(out=khat[:, :, :, 0:32],
                                        in0=pbk[:, :, :, 32:64],
                                        in1=skN[:, :, :, 0:32], op=OP.mult)
                nc.vector.tensor_tensor(out=khat[:, :, :, 32:64],
                                        in0=pbk[:, :, :, 0:32],
                                        in1=skN[:, :, :, 32:64], op=OP.mult)
                nc.vector.tensor_tensor(out=khat, in0=khat, in1=r1k, op=OP.add)
                rskb = rsk.unsqueeze(3).broadcast_to([128, 16, KVH, D])
                nc.vector.tensor_tensor(out=khat[:, 0:8], in0=khat[:, 0:8],
                                        in1=rskb[:, 0:8], op=OP.mult)
                nc.gpsimd.tensor_tensor(out=khat[:, 8:16], in0=khat[:, 8:16],
                                        in1=rskb[:, 8:16], op=OP.mult)
                for mt in range(16):
                    nc.sync.dma_start_transpose(
                        out=kT_sb[:, :, mt * 128 : (mt + 1) * 128],
                        in_=khat[:, mt])

                # ---- batched Q epilogue (per 8-head half)
                for nn in range(2):
                    pbn, qhn, r1n = pbq[:, nn], qhat[:, nn], r1q[:, nn]
                    nrmq = stage.tile([128, 4, H // 2], f32, tag="nrmq")
                    nc.scalar.activation(nrmq, ssq_sb[:, nn], AF.Sqrt)
                    rsq = stage.tile([128, 4, H // 2], f32, tag="rsq")
                    nc.vector.reciprocal(rsq, nrmq)
                    if ht == 1:
                        cqN = tqc_sb.broadcast_to([128, 4, H // 2, D])
                        sqN = tqs_sb.broadcast_to([128, 4, H // 2, D])
                    else:
                        cqN = tqc_sb[:, :, nn * 8 : (nn + 1) * 8, :]
                        sqN = tqs_sb[:, :, nn * 8 : (nn + 1) * 8, :]
                    nc.vector.tensor_tensor(out=r1n, in0=pbn, in1=cqN,
                                            op=OP.mult)
                    nc.vector.tensor_tensor(out=qhn[:, :, :, 0:32],
                                            in0=pbn[:, :, :, 32:64],
                                            in1=sqN[:, :, :, 0:32], op=OP.mult)
                    nc.vector.tensor_tensor(out=qhn[:, :, :, 32:64],
                                            in0=pbn[:, :, :, 0:32],
                                            in1=sqN[:, :, :, 32:64],
                                            op=OP.mult)
                    nc.vector.tensor_tensor(out=qhn, in0=qhn, in1=r1n,
                                            op=OP.add)
                    rsqb = rsq.unsqueeze(3).broadcast_to([128, 4, H // 2, D])
                    nc.vector.tensor_tensor(out=qhn[:, 0:2], in0=qhn[:, 0:2],
                                            in1=rsqb[:, 0:2], op=OP.mult)
                    nc.gpsimd.tensor_tensor(out=qhn[:, 2:4], in0=qhn[:, 2:4],
                                            in1=rsqb[:, 2:4], op=OP.mult)
                    for m in range(4):
                        nc.sync.dma_start_transpose(
                            out=qT_sb[:, nn * 4 : (nn + 1) * 4,
                                      m * 128 : (m + 1) * 128],
                            in_=qhat[:, nn, m])
            ap_ctx.__exit__(None, None, None)
            xp_ctx.__exit__(None, None, None)
            if 'B' not in PHASES:
                continue

            # ---- B: attention per head (wo weights + residual stream in
            # concurrently). exp: PSUM fp32 scores -> fp8-e5m2 probabilities
            # in one op per 256-key block, rotated across ScalarE/DVE/GpSimd.
            wo_ctx = tc.tile_pool(name="wo_pool", bufs=1)
            wo_pool = wo_ctx.__enter__()
            wo_sb = wo_pool.tile([128, 8, DIM], e4)
            qxr_sb = wo_pool.tile([128, 4, DIM], f32)
            wo_pool.seal()
            nc.sync.dma_start(out=wo_sb, in_=wo8_d[:, :, :])
            nc.sync.dma_start(out=qxr_sb, in_=qxr_d[:, :, :])
            with tc.tile_pool(name="sT_ps", bufs=3, space="PSUM") as sT_ps, \
                 tc.tile_pool(name="oT_ps", bufs=2, space="PSUM") as oT_ps, \
                 tc.tile_pool(name="pT_pool", bufs=4) as pT_pool, \
                 tc.tile_pool(name="small", bufs=2) as small:
                for h in range(H):
                    kvh = h % KVH
                    jq, qp_off = h // 2, 64 * (h % 2)
                    ktile, kp_off = kvh // 2, 64 * (kvh % 2)
                    oT = oT_ps.tile([128, 512], f32)

                    def do_av(pT8, s):
                        nc.tensor.matmul(
                            oT,
                            lhsT=v4[:, 2 * s : 2 * s + 2, kvh, :],
                            rhs=pT8,
                            start=(s == 0), stop=(s == 7), perf_mode=DR)

                    pending = None
                    for s in range(8):
                        sT = sT_ps.tile([128, 2, 512], f32)
                        for i in range(2):
                            kt = 2 * s + i
                            nc.tensor.matmul(
                                sT[:, i, :],
                                lhsT=kT_sb[kp_off : kp_off + 64, ktile,
                                           kt * 128 : (kt + 1) * 128],
                                rhs=qT_sb[qp_off : qp_off + 64, jq, :],
                                start=True, stop=True)
                        pT8 = pT_pool.tile([128, 2, 512], e5)
                        slot = ROT[(h * 8 + s) % 16]
                        if slot == 'A':
                            nc.scalar.activation(pT8, sT, AF.Exp,
                                                 scale=ACT_EXP_SCALE)
                        else:
                            nc.vector.tensor_scalar(
                                out=pT8.bitcast(u8), in0=sT,
                                scalar1=SCH_E5_SLOPE, scalar2=SCH_E5_BIAS,
                                op0=OP.mult, op1=OP.add)
                        if pending is not None:
                            do_av(*pending)
                        pending = (pT8, s)
                    do_av(*pending)
                    recip = small.tile([1, 512], f32, tag="recip")
                    nc.vector.reciprocal(recip, oT[D : D + 1, :])
                    rb = small.tile([D, 512], f32, tag="rb")
                    nc.gpsimd.partition_broadcast(rb, recip)
                    nc.vector.tensor_tensor(
                        out=aoT_sb[qp_off : qp_off + 64, jq, :],
                        in0=oT[0:D, :], in1=rb, op=OP.mult)

            # ---- C: output projection (natural orientation), 1/64 de-scale
            # + residual(+bias) add, store
            if 'C' not in PHASES:
                wo_ctx.__exit__(None, None, None)
                continue
            with tc.tile_pool(name="y_ps", bufs=2, space="PSUM") as y_ps, \
                 tc.tile_pool(name="ystage", bufs=2) as ystage:
                for qt in range(4):
                    yp = y_ps.tile([128, 2, 512], f32)
                    for ch in range(2):
                        for s in range(4):
                            nc.tensor.matmul(
                                yp[:, ch, :],
                                lhsT=aoT_sb[:, 2 * s : 2 * s + 2,
                                            qt * 128 : (qt + 1) * 128],
                                rhs=wo_sb[:, 2 * s : 2 * s + 2,
                                          ch * 512 : (ch + 1) * 512],
                                start=(s == 0), stop=(s == 3), perf_mode=DR)
                    ysc = ystage.tile([128, DIM], f32, tag="ysc")
                    nc.scalar.mul(ysc, yp.rearrange("p a b -> p (a b)"),
                                  1.0 / 64.0)
                    yn = ystage.tile([128, DIM], f32, tag="yn")
                    nc.vector.tensor_tensor(out=yn, in0=ysc,
                                            in1=qxr_sb[:, qt, :], op=OP.add)
                    nc.sync.dma_start(out=y_d[qt * 128 : (qt + 1) * 128, :],
                                      in_=yn)
            wo_ctx.__exit__(None, None, None)

    nc.compile()
    return nc


def _get_nc(ht, htk, repeat=1):
    key = (ht, htk, repeat, PHASES)
    if key not in _CACHE:
        _CACHE[key] = _build_nc(ht, htk, repeat)
    return _CACHE[key]


# ---------------------------------------------------------------- entry point
def make_in_maps(x, Wq, Wkv, q_gamma, k_gamma, Wo, bo):
    import ml_dtypes
    bf = ml_dtypes.bfloat16
    e4 = ml_dtypes.float8_e4m3
    x = np.ascontiguousarray(np.asarray(x, dtype=np.float32))
    Wq64 = (np.asarray(Wq, dtype=np.float32) * 64.0).astype(e4)
    Wkv64 = (np.asarray(Wkv, dtype=np.float32) * 64.0).astype(e4)
    Wo64 = (np.asarray(Wo, dtype=np.float32) * 64.0).astype(e4)
    bo = np.asarray(bo, dtype=np.float32)
    qg = np.asarray(q_gamma, dtype=np.float64).reshape(H, D)
    kg = np.asarray(k_gamma, dtype=np.float64).reshape(KVH, D)

    ht = 1 if np.all(qg == 1.0) else H
    htk = 1 if np.all(kg == 1.0) else KVH

    def ttiles(a):  # [n, h, d] bf16 -> [128, n//128, h, d]
        n, h, d = a.shape
        return np.ascontiguousarray(
            a.astype(bf).reshape(n // 128, 128, h, d).transpose(1, 0, 2, 3))

    def ktiles(w):  # [dim, cols] fp8 -> [128, dim//128, cols]
        dim, cols = w.shape
        return np.ascontiguousarray(
            w.reshape(dim // 128, 128, cols).transpose(1, 0, 2))

    pos = np.arange(N)
    tkc, tks = _make_tables(pos, -1.0, kg[:htk])
    tkc_t, tks_t = ttiles(tkc), ttiles(tks)
    wq_t, wkv_t, wo_t = ktiles(Wq64), ktiles(Wkv64), ktiles(Wo64)

    in_maps = []
    for c in range(NCORES):
        bi, qi = c // 4, c % 4
        xT8 = ktiles(np.ascontiguousarray(x[bi].T).astype(e4))  # [128, 8, N]
        qpos = pos[qi * QS : (qi + 1) * QS]
        tqc, tqs = _make_tables(qpos, +1.0, qg[:ht])
        qxr = x[bi, qi * QS : (qi + 1) * QS] + bo[None, :]
        qxr_t = np.ascontiguousarray(
            qxr.reshape(4, 128, DIM).transpose(1, 0, 2))
        in_maps.append({
            "xT8": xT8,
            "qxT8": np.ascontiguousarray(xT8[:, :, qi * QS : (qi + 1) * QS]),
            "wq8": wq_t, "wkv8": wkv_t, "wo8": wo_t,
            "qxr": qxr_t,
            "tqc": ttiles(tqc), "tqs": ttiles(tqs),
            "tkc": tkc_t, "tks": tks_t,
        })
    return in_maps, (ht, htk)


def kernel(x, Wq, Wkv, q_gamma, k_gamma, Wo, bo):
    from concourse import bass_utils

    in_maps, (ht, htk) = make_in_maps(x, Wq, Wkv, q_gamma, k_gamma, Wo, bo)
    nc = _get_nc(ht, htk)
    res = bass_utils.run_bass_kernel_spmd(nc, in_maps,
                                          core_ids=list(range(NCORES)))
    out = np.zeros((B, N, DIM), np.float32)
    for c in range(NCORES):
        bi, qi = c // 4, c % 4
        out[bi, qi * QS : (qi + 1) * QS] = res.results[c]["y"]
    return out
